# revision 13
# baseline (speedup 1.0000x reference)
"""Trainium2 Bass kernel for nn_Attention_84516366450883 (gnn message passing).

Computation (reference):
    leave_emb = W_emb[leaves]          # [N, A, E]
    anc_emb   = W_emb[ancestors]       # [N, A, E]
    mlp  = tanh(concat(leave_emb, anc_emb) @ W_attention + b)   # [N, A, ATT]
    pre  = mlp @ v                     # [N, A]
    attn = softmax(pre, axis=1)
    out  = einsum('nae,na->ne', anc_emb, attn)                  # [N, E]

Sharding: data-parallel over N across 8 cores; params replicated; no
collectives.

Gather strategy: the HW indirect-DMA path consumes ONE offset per dest
partition (multi-offset gathers silently read consecutive rows), and issuing
16 per-slot indirect DMAs per tile pays ~1us SWDGE descriptor-generation
fixed cost each (the old 2.3ms bottleneck). Instead we use the custom
`dma_gather` instruction (int16 indices): the host groups each core's work
into chunks of 16 tiles (= 32768 gathered rows), compacts the <=32768
distinct embedding rows of each chunk into a per-chunk table, and remaps
indices to int16. One dma_gather(transpose=True) per tile then fetches all
2048 rows AND delivers them emb-major ([emb, 16*128] in SBUF) -- no PE
transposes, no PSUM round-trip.

Per-core dataflow per tile (128 codes):
  - dma_gather -> gt[emb, 16*128] bf16 (leaf slots 0-7, anc slots 8-15)
  - z[att, 1024] = W_l.T @ LT + W_a.T @ AT  (4 bf16 matmuls, free=512)
  - mlp = tanh(z + b) on ACT (bf16)
  - pre[codes, j] = mlp_j.T @ v  (8 tiny matmuls -> [128, 8] PSUM)
  - softmax: ACT exp with fused row-sum accumulator, DVE recip + scale
  - attn.T via one PE transpose ([128,8] -> [8,128])
  - weighted sum in emb-major space: 8 DVE muls with partition-broadcast
    attn rows + grouped DVE reduce -> outT[emb, codes] f32
  - store outT tile; host un-transposes the final [E, nsh] -> [nsh, E]
The loop is software-pipelined: softmax/weighted-sum of tile t-1 overlap
the gather of tile t.
"""

import sys

if "/opt/trn_rl_repo" not in sys.path:
    sys.path.insert(0, "/opt/trn_rl_repo")

import numpy as np

VOCAB, EMB, ATT = 100000, 128, 128
N_CODES, N_ANC = 100000, 8
NCORES = 8
NSH = N_CODES // NCORES            # 12500 codes per core
TILES = (NSH + 127) // 128         # 98
NPAD = TILES * 128                 # 12544
NSLOT = 2 * N_ANC                  # 16 gathered rows per code
GROUP_TILES = 16                   # tiles per compacted gather table
TAB_ROWS = GROUP_TILES * 128 * NSLOT  # 32768: max distinct rows per group

_nc_cache = {}


def _build(tiles=TILES, num_devices=NCORES):
    import concourse.bacc as bacc
    import concourse.tile as tile
    from concourse import bass, mybir
    from concourse.masks import make_identity

    f32 = mybir.dt.float32
    bf16 = mybir.dt.bfloat16
    i16 = mybir.dt.int16
    Act = mybir.ActivationFunctionType
    groups = (tiles + GROUP_TILES - 1) // GROUP_TILES
    idxcols = 128 * NSLOT // 16    # 128 wrapped-int16 columns per tile

    nc = bacc.Bacc("TRN2", target_bir_lowering=False, debug=False,
                   num_devices=num_devices)
    tab = nc.dram_tensor("tab", (groups * TAB_ROWS, EMB), bf16,
                         kind="ExternalInput").ap()
    w_att = nc.dram_tensor("w_att", (2 * EMB, ATT), bf16, kind="ExternalInput").ap()
    b_att = nc.dram_tensor("b_att", (1, ATT), f32, kind="ExternalInput").ap()
    v_att = nc.dram_tensor("v_att", (1, ATT), bf16, kind="ExternalInput").ap()
    # wrapped dma_gather index layout, replicated across the 8 Q7 cores'
    # 16-partition groups: idx16[16c + r, t*128 + q] = remapped id of the
    # (q*16 + r)-th gathered row of tile t
    idx = nc.dram_tensor("idx", (128, tiles * idxcols), i16,
                         kind="ExternalInput").ap()
    outT = nc.dram_tensor("outT", (EMB, tiles * 128), f32,
                          kind="ExternalOutput").ap()

    with tile.TileContext(nc) as tc:
        with (
            tc.tile_pool(name="const", bufs=1) as cpool,
            tc.tile_pool(name="gat", bufs=3) as gpool,
            tc.tile_pool(name="mlp", bufs=2) as mpool,
            tc.tile_pool(name="sm", bufs=2) as smpool,
            tc.tile_pool(name="at", bufs=2) as apool,
            tc.tile_pool(name="ws", bufs=2) as wpool,
            tc.tile_pool(name="st", bufs=3) as stpool,
            tc.tile_pool(name="psz", bufs=1, space="PSUM") as psz_pool,
            tc.tile_pool(name="psb", bufs=2, space="PSUM") as psb_pool,
            tc.tile_pool(name="pss", bufs=1, space="PSUM") as pss_pool,
        ):
            idx_sb = cpool.tile([128, tiles * idxcols], i16)
            nc.sync.dma_start(idx_sb[:], idx)
            ident = cpool.tile([128, 128], bf16)
            make_identity(nc, ident[:])
            # E[c, a*128+p] = (a == c): selector masks so bc_a = E_a.T @ attnT
            # replicates attnT row a across all 128 partitions
            emask = cpool.tile([N_ANC, N_ANC * 128], bf16)
            nc.gpsimd.memset(emask[:], 1.0)
            nc.gpsimd.affine_select(
                emask[:], emask[:], pattern=[[1, N_ANC * 128]],
                compare_op=mybir.AluOpType.is_ge, fill=0.0,
                base=0, channel_multiplier=-128)
            nc.gpsimd.affine_select(
                emask[:], emask[:], pattern=[[-1, N_ANC * 128]],
                compare_op=mybir.AluOpType.is_ge, fill=0.0,
                base=127, channel_multiplier=128)
            wl = cpool.tile([EMB, ATT], bf16)
            nc.sync.dma_start(wl[:], w_att[0:EMB, :])
            wa = cpool.tile([EMB, ATT], bf16)
            nc.sync.dma_start(wa[:], w_att[EMB:2 * EMB, :])
            bias = cpool.tile([ATT, 1], f32)
            nc.sync.dma_start(bias[:], b_att.rearrange("a b -> b a"))
            vv = cpool.tile([ATT, 1], bf16)
            nc.sync.dma_start(vv[:], v_att.rearrange("a b -> b a"))

            prev = None  # (gt, mlp) of tile t-1

            for t in range(tiles + 1):
                if t < tiles:
                    g = t // GROUP_TILES
                    gt = gpool.tile([128, NSLOT * EMB], bf16, tag="gt")
                    nc.gpsimd.dma_gather(
                        out_ap=gt[:].rearrange("p (one n) -> p one n", one=1),
                        in_ap=tab[g * TAB_ROWS:(g + 1) * TAB_ROWS, :],
                        idxs_ap=idx_sb[:, t * idxcols:(t + 1) * idxcols],
                        num_idxs=NSLOT * 128,
                        num_idxs_reg=NSLOT * 128,
                        elem_size=EMB,
                        transpose=True,
                        single_packet=False,
                    )

                # --- stage 2 for tile t-1 ------------------------------
                if prev is not None:
                    pgt, pmlp = prev
                    s = t - 1
                    pre = pss_pool.tile([128, N_ANC], f32, tag="pre")
                    for j in range(N_ANC):
                        nc.tensor.matmul(pre[:, j:j + 1],
                                         lhsT=pmlp[:, j * ATT:(j + 1) * ATT],
                                         rhs=vv[:], start=True, stop=True)
                    ex = smpool.tile([128, N_ANC], f32, tag="ex")
                    ssum = smpool.tile([128, 1], f32, tag="ssum")
                    nc.scalar.activation(ex[:], pre[:], Act.Exp,
                                         accum_out=ssum[:])
                    rec = smpool.tile([128, 1], f32, tag="rec")
                    nc.vector.reciprocal(rec[:], ssum[:])
                    attn = smpool.tile([128, N_ANC], bf16, tag="attn")
                    nc.vector.tensor_mul(attn[:], ex[:],
                                         rec[:].to_broadcast([128, N_ANC]))
                    # attn.T -> [8, 128] so weighted sum can broadcast rows
                    pT = pss_pool.tile([N_ANC, 128], bf16, tag="pT")
                    nc.tensor.transpose(pT[:], attn[:], ident[:])
                    attnT = apool.tile([N_ANC, 128], bf16, tag="attnT")
                    nc.scalar.copy(attnT[:], pT[:])
                    # replicate each attn row across all 128 partitions via
                    # K=1 PE matmuls (DVE can't partition-broadcast)
                    bc = psb_pool.tile([128, N_ANC * 128], f32, tag="bc")
                    for a in range(N_ANC):
                        nc.tensor.matmul(bc[:, a * 128:(a + 1) * 128],
                                         lhsT=emask[:, a * 128:(a + 1) * 128],
                                         rhs=attnT[:], start=True, stop=True)
                    # weighted sum over ancestors, emb-major
                    ws = wpool.tile([128, N_ANC * EMB], bf16, tag="ws")
                    nc.vector.tensor_mul(ws[:], pgt[:, N_ANC * EMB:NSLOT * EMB],
                                         bc[:])
                    stage = stpool.tile([128, EMB], f32, tag="stage")
                    nc.vector.tensor_reduce(
                        stage[:], ws[:].rearrange("p (a n) -> p n a", a=N_ANC),
                        axis=mybir.AxisListType.X, op=mybir.AluOpType.add)
                    nc.sync.dma_start(outT[:, s * 128:(s + 1) * 128], stage[:])

                if t < tiles:
                    # --- z = W_l.T @ LT + W_a.T @ AT ----------------------
                    z = psz_pool.tile([128, N_ANC * ATT], f32, tag="z")
                    nc.tensor.matmul(z[:, 0:512], lhsT=wl[:], rhs=gt[:, 0:512],
                                     start=True, stop=False)
                    nc.tensor.matmul(z[:, 0:512], lhsT=wa[:], rhs=gt[:, 1024:1536],
                                     start=False, stop=True)
                    nc.tensor.matmul(z[:, 512:1024], lhsT=wl[:],
                                     rhs=gt[:, 512:1024], start=True, stop=False)
                    nc.tensor.matmul(z[:, 512:1024], lhsT=wa[:],
                                     rhs=gt[:, 1536:2048], start=False, stop=True)
                    mlp = mpool.tile([128, N_ANC * ATT], bf16, tag="mlp")
                    nc.scalar.activation(mlp[:, 0:512], z[:, 0:512], Act.Tanh,
                                         bias=bias[:])
                    nc.scalar.activation(mlp[:, 512:1024], z[:, 512:1024],
                                         Act.Tanh, bias=bias[:])
                    prev = (gt, mlp)

    nc.compile()
    return nc


def _get_nc(tiles=TILES, num_devices=NCORES):
    key = (tiles, num_devices)
    if key not in _nc_cache:
        _nc_cache[key] = _build(tiles, num_devices)
    return _nc_cache[key]


def _prep_core(ids_pad, W_bf16, tiles=TILES):
    """ids_pad: [tiles*128, 16] int32. Returns (tab, idx16) for one core."""
    groups = (tiles + GROUP_TILES - 1) // GROUP_TILES
    idxcols = 128 * NSLOT // 16
    tab = np.zeros((groups * TAB_ROWS, EMB), dtype=W_bf16.dtype)
    idx16 = np.zeros((16, tiles * idxcols), dtype=np.int16)
    for g in range(groups):
        t0, t1 = g * GROUP_TILES, min(tiles, (g + 1) * GROUP_TILES)
        codes = ids_pad[t0 * 128:t1 * 128]               # [nt*128, 16]
        nt = t1 - t0
        # gather order within tile: k = s*128 + n
        korder = codes.reshape(nt, 128, NSLOT).transpose(0, 2, 1)  # [nt, s, n]
        flat = korder.reshape(-1)                         # nt*2048, k-major
        uniq, inv = np.unique(flat, return_inverse=True)
        tab[g * TAB_ROWS:g * TAB_ROWS + len(uniq)] = W_bf16[uniq]
        inv = inv.astype(np.int16).reshape(nt, NSLOT * 128)
        for ti in range(nt):
            # wrapped layout: idxs[k%16, k//16] = inv[k]
            blk = inv[ti].reshape(idxcols, 16).T          # [16, 128]
            idx16[:, (t0 + ti) * idxcols:(t0 + ti + 1) * idxcols] = blk
    return tab, np.ascontiguousarray(np.tile(idx16, (8, 1)))


def _prep_in_maps(inputs, tiles=TILES):
    import ml_dtypes

    W_emb = np.ascontiguousarray(
        np.asarray(inputs["W_emb"], dtype=np.float32).astype(ml_dtypes.bfloat16))
    W_attention = np.ascontiguousarray(
        np.asarray(inputs["W_attention"], dtype=np.float32).astype(ml_dtypes.bfloat16))
    b_attention = np.ascontiguousarray(
        np.asarray(inputs["b_attention"], dtype=np.float32).reshape(1, ATT))
    v_attention = np.ascontiguousarray(
        np.asarray(inputs["v_attention"],
                   dtype=np.float32).astype(ml_dtypes.bfloat16).reshape(1, ATT))
    leaves = np.asarray(inputs["leaves"]).astype(np.int32)
    ancestors = np.asarray(inputs["ancestors"]).astype(np.int32)

    idx_all = np.concatenate([leaves, ancestors], axis=1)   # [N, 16]
    npad = tiles * 128
    in_maps = []
    for c in range(NCORES):
        shard = idx_all[c * NSH:(c + 1) * NSH]
        pad = np.zeros((npad, NSLOT), dtype=np.int32)
        pad[:NSH] = shard
        tab, idx16 = _prep_core(pad, W_emb, tiles)
        in_maps.append({
            "tab": tab,
            "w_att": W_attention,
            "b_att": b_attention,
            "v_att": v_attention,
            "idx": idx16,
        })
    return in_maps


def run(inputs, trace=False, **kwargs):
    """Run on the 8 NeuronCores; returns (output [N, E] f32, BassKernelResults)."""
    from concourse import bass_utils
    nc = _get_nc()
    in_maps = _prep_in_maps(inputs)
    res = bass_utils.run_bass_kernel_spmd(
        nc, in_maps, core_ids=list(range(NCORES)), trace=trace, **kwargs)
    outs = [res.results[c]["outT"][:, :NSH].T for c in range(NCORES)]
    full = np.concatenate(outs, axis=0).astype(np.float32)
    return full, res


def kernel(**inputs) -> np.ndarray:
    full, _ = run(inputs, trace=False)
    return full


# revision 19
# speedup vs baseline: 1.1583x; 1.1583x over previous
"""Trainium2 Bass kernel for nn_Attention_84516366450883 (gnn message passing).

Computation (reference):
    leave_emb = W_emb[leaves]          # [N, A, E]
    anc_emb   = W_emb[ancestors]       # [N, A, E]
    mlp  = tanh(concat(leave_emb, anc_emb) @ W_attention + b)   # [N, A, ATT]
    pre  = mlp @ v                     # [N, A]
    attn = softmax(pre, axis=1)
    out  = einsum('nae,na->ne', anc_emb, attn)                  # [N, E]

Sharding: data-parallel over N across 8 cores; params replicated; no
collectives.

Gather strategy: the HW indirect-DMA path consumes ONE offset per dest
partition (multi-offset gathers silently read consecutive rows), and issuing
16 per-slot indirect DMAs per tile pays ~1us SWDGE descriptor-generation
fixed cost each (the old 2.3ms bottleneck). Instead we use the custom
`dma_gather` instruction (int16 indices): the host groups each core's work
into chunks of 16 tiles (= 32768 gathered rows), compacts the <=32768
distinct embedding rows of each chunk into a per-chunk table, and remaps
indices to int16. One dma_gather(transpose=True) per tile then fetches all
2048 rows AND delivers them emb-major ([emb, 16*128] in SBUF) -- no PE
transposes, no PSUM round-trip.

Per-core dataflow per tile (128 codes):
  - dma_gather -> gt[emb, 16*128] bf16 (leaf slots 0-7, anc slots 8-15)
  - z[att, 1024] = W_l.T @ LT + W_a.T @ AT  (4 bf16 matmuls, free=512)
  - mlp = tanh(z + b) on ACT (bf16)
  - pre[codes, j] = mlp_j.T @ v  (8 tiny matmuls -> [128, 8] PSUM)
  - softmax: ACT exp with fused row-sum accumulator, DVE recip + scale
  - attn.T via one PE transpose ([128,8] -> [8,128])
  - weighted sum in emb-major space: 8 DVE muls with partition-broadcast
    attn rows + grouped DVE reduce -> outT[emb, codes] f32
  - store outT tile; host un-transposes the final [E, nsh] -> [nsh, E]
The loop is software-pipelined: softmax/weighted-sum of tile t-1 overlap
the gather of tile t.
"""

import sys

if "/opt/trn_rl_repo" not in sys.path:
    sys.path.insert(0, "/opt/trn_rl_repo")

import numpy as np

VOCAB, EMB, ATT = 100000, 128, 128
N_CODES, N_ANC = 100000, 8
NCORES = 8
NSH = N_CODES // NCORES            # 12500 codes per core
TILES = (NSH + 127) // 128         # 98
NPAD = TILES * 128                 # 12544
NSLOT = 2 * N_ANC                  # 16 gathered rows per code
GROUP_TILES = 16                   # tiles per compacted gather table
TAB_ROWS = GROUP_TILES * 128 * NSLOT  # 32768: max distinct rows per group
GATHER_TILES = 2                   # tiles fetched per dma_gather instruction

_nc_cache = {}


def _build(tiles=TILES, num_devices=NCORES):
    import concourse.bacc as bacc
    import concourse.tile as tile
    from concourse import bass, mybir
    from concourse.masks import make_identity

    f32 = mybir.dt.float32
    bf16 = mybir.dt.bfloat16
    i16 = mybir.dt.int16
    Act = mybir.ActivationFunctionType
    groups = (tiles + GROUP_TILES - 1) // GROUP_TILES
    idxcols = 128 * NSLOT // 16    # 128 wrapped-int16 columns per tile

    nc = bacc.Bacc("TRN2", target_bir_lowering=False, debug=False,
                   num_devices=num_devices)
    tab = nc.dram_tensor("tab", (groups * TAB_ROWS, EMB), bf16,
                         kind="ExternalInput").ap()
    w_att = nc.dram_tensor("w_att", (2 * EMB, ATT), bf16, kind="ExternalInput").ap()
    b_att = nc.dram_tensor("b_att", (1, ATT), f32, kind="ExternalInput").ap()
    v_att = nc.dram_tensor("v_att", (1, ATT), bf16, kind="ExternalInput").ap()
    # wrapped dma_gather index layout, one [128, GATHER_TILES*idxcols] block
    # per gather instruction, replicated across the 8 Q7 cores' 16-partition
    # groups: block[16c + r, q] = remapped id of the (q*16 + r)-th row
    assert tiles % GATHER_TILES == 0
    ngath = tiles // GATHER_TILES
    gcols = GATHER_TILES * idxcols
    idx = nc.dram_tensor("idx", (ngath, 128, gcols), i16,
                         kind="ExternalInput").ap()
    outT = nc.dram_tensor("outT", (EMB, tiles * 128), f32,
                          kind="ExternalOutput").ap()

    with tile.TileContext(nc) as tc:
        with (
            tc.tile_pool(name="const", bufs=1) as cpool,
            tc.tile_pool(name="idxp", bufs=3) as ipool,
            tc.tile_pool(name="gat", bufs=2) as gpool,
            tc.tile_pool(name="mlp", bufs=2) as mpool,
            tc.tile_pool(name="sm", bufs=2) as smpool,
            tc.tile_pool(name="at", bufs=2) as apool,
            tc.tile_pool(name="ws", bufs=2) as wpool,
            tc.tile_pool(name="st", bufs=3) as stpool,
            tc.tile_pool(name="psz", bufs=1, space="PSUM") as psz_pool,
            tc.tile_pool(name="psb", bufs=2, space="PSUM") as psb_pool,
            tc.tile_pool(name="pss", bufs=1, space="PSUM") as pss_pool,
        ):
            ident = cpool.tile([128, 128], bf16)
            make_identity(nc, ident[:])
            # E[c, a*128+p] = (a == c): selector masks so bc_a = E_a.T @ attnT
            # replicates attnT row a across all 128 partitions
            emask = cpool.tile([N_ANC, N_ANC * 128], bf16)
            nc.gpsimd.memset(emask[:], 1.0)
            nc.gpsimd.affine_select(
                emask[:], emask[:], pattern=[[1, N_ANC * 128]],
                compare_op=mybir.AluOpType.is_ge, fill=0.0,
                base=0, channel_multiplier=-128)
            nc.gpsimd.affine_select(
                emask[:], emask[:], pattern=[[-1, N_ANC * 128]],
                compare_op=mybir.AluOpType.is_ge, fill=0.0,
                base=127, channel_multiplier=128)
            wl = cpool.tile([EMB, ATT], bf16)
            nc.sync.dma_start(wl[:], w_att[0:EMB, :])
            wa = cpool.tile([EMB, ATT], bf16)
            nc.sync.dma_start(wa[:], w_att[EMB:2 * EMB, :])
            bias = cpool.tile([ATT, 1], f32)
            nc.sync.dma_start(bias[:], b_att.rearrange("a b -> b a"))
            vv = cpool.tile([ATT, 1], bf16)
            nc.sync.dma_start(vv[:], v_att.rearrange("a b -> b a"))

            prev = None  # (gt-slice, mlp) of tile t-1
            gt2 = None

            for t in range(tiles + 1):
                if t < tiles and t % GATHER_TILES == 0:
                    g = t // GROUP_TILES
                    gi = t // GATHER_TILES
                    nidx = GATHER_TILES * NSLOT * 128
                    idx_sb = ipool.tile([128, gcols], i16, tag="idx")
                    nc.sync.dma_start(idx_sb[:], idx[gi])
                    gt2 = gpool.tile([128, GATHER_TILES * NSLOT * EMB], bf16,
                                     tag="gt")
                    nc.gpsimd.dma_gather(
                        out_ap=gt2[:].rearrange("p (one n) -> p one n", one=1),
                        in_ap=tab[g * TAB_ROWS:(g + 1) * TAB_ROWS, :],
                        idxs_ap=idx_sb[:],
                        num_idxs=nidx,
                        num_idxs_reg=nidx,
                        elem_size=EMB,
                        transpose=True,
                        single_packet=False,
                    )
                if t < tiles:
                    off = (t % GATHER_TILES) * NSLOT * EMB
                    gt = gt2[:, off:off + NSLOT * EMB]

                # --- stage 2 for tile t-1 ------------------------------
                if prev is not None:
                    pgt, pmlp = prev
                    s = t - 1
                    pre = pss_pool.tile([128, N_ANC], f32, tag="pre")
                    for j in range(N_ANC):
                        nc.tensor.matmul(pre[:, j:j + 1],
                                         lhsT=pmlp[:, j * ATT:(j + 1) * ATT],
                                         rhs=vv[:], start=True, stop=True)
                    ex = smpool.tile([128, N_ANC], f32, tag="ex")
                    ssum = smpool.tile([128, 1], f32, tag="ssum")
                    nc.scalar.activation(ex[:], pre[:], Act.Exp,
                                         accum_out=ssum[:])
                    rec = smpool.tile([128, 1], f32, tag="rec")
                    nc.vector.reciprocal(rec[:], ssum[:])
                    attn = smpool.tile([128, N_ANC], bf16, tag="attn")
                    nc.vector.tensor_mul(attn[:], ex[:],
                                         rec[:].to_broadcast([128, N_ANC]))
                    # attn.T -> [8, 128] so weighted sum can broadcast rows
                    pT = pss_pool.tile([N_ANC, 128], bf16, tag="pT")
                    nc.tensor.transpose(pT[:], attn[:], ident[:])
                    attnT = apool.tile([N_ANC, 128], bf16, tag="attnT")
                    nc.scalar.copy(attnT[:], pT[:])
                    # replicate each attn row across all 128 partitions via
                    # K=1 PE matmuls (DVE can't partition-broadcast)
                    bc = psb_pool.tile([128, N_ANC * 128], f32, tag="bc")
                    for a in range(N_ANC):
                        nc.tensor.matmul(bc[:, a * 128:(a + 1) * 128],
                                         lhsT=emask[:, a * 128:(a + 1) * 128],
                                         rhs=attnT[:], start=True, stop=True)
                    # weighted sum over ancestors, emb-major
                    ws = wpool.tile([128, N_ANC * EMB], bf16, tag="ws")
                    nc.vector.tensor_mul(ws[:], pgt[:, N_ANC * EMB:NSLOT * EMB],
                                         bc[:])
                    stage = stpool.tile([128, EMB], f32, tag="stage")
                    nc.vector.tensor_reduce(
                        stage[:], ws[:].rearrange("p (a n) -> p n a", a=N_ANC),
                        axis=mybir.AxisListType.X, op=mybir.AluOpType.add)
                    nc.sync.dma_start(outT[:, s * 128:(s + 1) * 128], stage[:])

                if t < tiles:
                    # --- z = W_l.T @ LT + W_a.T @ AT ----------------------
                    z = psz_pool.tile([128, N_ANC * ATT], f32, tag="z")
                    nc.tensor.matmul(z[:, 0:512], lhsT=wl[:], rhs=gt[:, 0:512],
                                     start=True, stop=False)
                    nc.tensor.matmul(z[:, 0:512], lhsT=wa[:], rhs=gt[:, 1024:1536],
                                     start=False, stop=True)
                    nc.tensor.matmul(z[:, 512:1024], lhsT=wl[:],
                                     rhs=gt[:, 512:1024], start=True, stop=False)
                    nc.tensor.matmul(z[:, 512:1024], lhsT=wa[:],
                                     rhs=gt[:, 1536:2048], start=False, stop=True)
                    mlp = mpool.tile([128, N_ANC * ATT], bf16, tag="mlp")
                    nc.scalar.activation(mlp[:, 0:512], z[:, 0:512], Act.Tanh,
                                         bias=bias[:])
                    nc.scalar.activation(mlp[:, 512:1024], z[:, 512:1024],
                                         Act.Tanh, bias=bias[:])
                    prev = (gt, mlp)

    nc.compile()
    return nc


def _get_nc(tiles=TILES, num_devices=NCORES):
    key = (tiles, num_devices)
    if key not in _nc_cache:
        _nc_cache[key] = _build(tiles, num_devices)
    return _nc_cache[key]


def _prep_core(ids_pad, W_bf16, tiles=TILES):
    """ids_pad: [tiles*128, 16] int32. Returns (tab, idx16) for one core."""
    groups = (tiles + GROUP_TILES - 1) // GROUP_TILES
    ngath = tiles // GATHER_TILES
    gcols = GATHER_TILES * 128 * NSLOT // 16
    tab = np.zeros((groups * TAB_ROWS, EMB), dtype=W_bf16.dtype)
    idx16 = np.zeros((ngath, 16, gcols), dtype=np.int16)
    for g in range(groups):
        t0, t1 = g * GROUP_TILES, min(tiles, (g + 1) * GROUP_TILES)
        codes = ids_pad[t0 * 128:t1 * 128]               # [nt*128, 16]
        nt = t1 - t0
        # gather order within tile: k = s*128 + n
        korder = codes.reshape(nt, 128, NSLOT).transpose(0, 2, 1)  # [nt, s, n]
        flat = korder.reshape(-1)                         # nt*2048, k-major
        uniq, inv = np.unique(flat, return_inverse=True)
        tab[g * TAB_ROWS:g * TAB_ROWS + len(uniq)] = W_bf16[uniq]
        inv = inv.astype(np.int16).reshape(nt // GATHER_TILES,
                                           GATHER_TILES * NSLOT * 128)
        for bi in range(nt // GATHER_TILES):
            # wrapped layout: idxs[k%16, k//16] = inv[k]
            gi = t0 // GATHER_TILES + bi
            idx16[gi] = inv[bi].reshape(gcols, 16).T      # [16, gcols]
    idx16 = np.broadcast_to(idx16[:, None, :, :],
                            (ngath, 8, 16, gcols)).reshape(ngath, 128, gcols)
    return tab, np.ascontiguousarray(idx16)


def _prep_in_maps(inputs, tiles=TILES):
    import ml_dtypes

    W_emb = np.ascontiguousarray(
        np.asarray(inputs["W_emb"], dtype=np.float32).astype(ml_dtypes.bfloat16))
    W_attention = np.ascontiguousarray(
        np.asarray(inputs["W_attention"], dtype=np.float32).astype(ml_dtypes.bfloat16))
    b_attention = np.ascontiguousarray(
        np.asarray(inputs["b_attention"], dtype=np.float32).reshape(1, ATT))
    v_attention = np.ascontiguousarray(
        np.asarray(inputs["v_attention"],
                   dtype=np.float32).astype(ml_dtypes.bfloat16).reshape(1, ATT))
    leaves = np.asarray(inputs["leaves"]).astype(np.int32)
    ancestors = np.asarray(inputs["ancestors"]).astype(np.int32)

    idx_all = np.concatenate([leaves, ancestors], axis=1)   # [N, 16]
    npad = tiles * 128
    in_maps = []
    for c in range(NCORES):
        shard = idx_all[c * NSH:(c + 1) * NSH]
        pad = np.zeros((npad, NSLOT), dtype=np.int32)
        pad[:NSH] = shard
        tab, idx16 = _prep_core(pad, W_emb, tiles)
        in_maps.append({
            "tab": tab,
            "w_att": W_attention,
            "b_att": b_attention,
            "v_att": v_attention,
            "idx": idx16,
        })
    return in_maps


def run(inputs, trace=False, **kwargs):
    """Run on the 8 NeuronCores; returns (output [N, E] f32, BassKernelResults)."""
    from concourse import bass_utils
    nc = _get_nc()
    in_maps = _prep_in_maps(inputs)
    res = bass_utils.run_bass_kernel_spmd(
        nc, in_maps, core_ids=list(range(NCORES)), trace=trace, **kwargs)
    outs = [res.results[c]["outT"][:, :NSH].T for c in range(NCORES)]
    full = np.concatenate(outs, axis=0).astype(np.float32)
    return full, res


def kernel(**inputs) -> np.ndarray:
    full, _ = run(inputs, trace=False)
    return full


# revision 20
# speedup vs baseline: 1.3293x; 1.1477x over previous
"""Trainium2 Bass kernel for nn_Attention_84516366450883 (gnn message passing).

Computation (reference):
    leave_emb = W_emb[leaves]          # [N, A, E]
    anc_emb   = W_emb[ancestors]       # [N, A, E]
    mlp  = tanh(concat(leave_emb, anc_emb) @ W_attention + b)   # [N, A, ATT]
    pre  = mlp @ v                     # [N, A]
    attn = softmax(pre, axis=1)
    out  = einsum('nae,na->ne', anc_emb, attn)                  # [N, E]

Sharding: data-parallel over N across 8 cores; params replicated; no
collectives.

Gather strategy: the HW indirect-DMA path consumes ONE offset per dest
partition (multi-offset gathers silently read consecutive rows), and issuing
16 per-slot indirect DMAs per tile pays ~1us SWDGE descriptor-generation
fixed cost each (the old 2.3ms bottleneck). Instead we use the custom
`dma_gather` instruction (int16 indices): the host groups each core's work
into chunks of 16 tiles (= 32768 gathered rows), compacts the <=32768
distinct embedding rows of each chunk into a per-chunk table, and remaps
indices to int16. One dma_gather(transpose=True) per tile then fetches all
2048 rows AND delivers them emb-major ([emb, 16*128] in SBUF) -- no PE
transposes, no PSUM round-trip.

Per-core dataflow per tile (128 codes):
  - dma_gather -> gt[emb, 16*128] bf16 (leaf slots 0-7, anc slots 8-15)
  - z[att, 1024] = W_l.T @ LT + W_a.T @ AT  (4 bf16 matmuls, free=512)
  - mlp = tanh(z + b) on ACT (bf16)
  - pre[codes, j] = mlp_j.T @ v  (8 tiny matmuls -> [128, 8] PSUM)
  - softmax: ACT exp with fused row-sum accumulator, DVE recip + scale
  - attn.T via one PE transpose ([128,8] -> [8,128])
  - weighted sum in emb-major space: 8 DVE muls with partition-broadcast
    attn rows + grouped DVE reduce -> outT[emb, codes] f32
  - store outT tile; host un-transposes the final [E, nsh] -> [nsh, E]
The loop is software-pipelined: softmax/weighted-sum of tile t-1 overlap
the gather of tile t.
"""

import sys

if "/opt/trn_rl_repo" not in sys.path:
    sys.path.insert(0, "/opt/trn_rl_repo")

import numpy as np

VOCAB, EMB, ATT = 100000, 128, 128
N_CODES, N_ANC = 100000, 8
NCORES = 8
NSH = N_CODES // NCORES            # 12500 codes per core
TILES = (NSH + 127) // 128         # 98
NPAD = TILES * 128                 # 12544
NSLOT = 2 * N_ANC                  # 16 gathered rows per code
GROUP_TILES = 16                   # tiles per compacted gather table
TAB_ROWS = GROUP_TILES * 128 * NSLOT  # 32768: max distinct rows per group
GATHER_TILES = 2                   # tiles fetched per dma_gather instruction

_nc_cache = {}


def _build(tiles=TILES, num_devices=NCORES):
    import concourse.bacc as bacc
    import concourse.tile as tile
    from concourse import bass, mybir
    from concourse.masks import make_identity

    f32 = mybir.dt.float32
    bf16 = mybir.dt.bfloat16
    i16 = mybir.dt.int16
    Act = mybir.ActivationFunctionType
    groups = (tiles + GROUP_TILES - 1) // GROUP_TILES
    idxcols = 128 * NSLOT // 16    # 128 wrapped-int16 columns per tile

    nc = bacc.Bacc("TRN2", target_bir_lowering=False, debug=False,
                   num_devices=num_devices)
    tab = nc.dram_tensor("tab", (groups * TAB_ROWS, EMB), bf16,
                         kind="ExternalInput").ap()
    w_att = nc.dram_tensor("w_att", (2 * EMB, ATT), bf16, kind="ExternalInput").ap()
    b_att = nc.dram_tensor("b_att", (1, ATT), f32, kind="ExternalInput").ap()
    v_att = nc.dram_tensor("v_att", (1, ATT), bf16, kind="ExternalInput").ap()
    # wrapped dma_gather index layout, one [128, GATHER_TILES*idxcols] block
    # per gather instruction, replicated across the 8 Q7 cores' 16-partition
    # groups: block[16c + r, q] = remapped id of the (q*16 + r)-th row
    assert tiles % GATHER_TILES == 0
    ngath = tiles // GATHER_TILES
    gcols = GATHER_TILES * idxcols
    idx = nc.dram_tensor("idx", (ngath, 128, gcols), i16,
                         kind="ExternalInput").ap()
    outT = nc.dram_tensor("outT", (EMB, tiles * 128), f32,
                          kind="ExternalOutput").ap()

    with tile.TileContext(nc) as tc:
        with (
            tc.tile_pool(name="const", bufs=1) as cpool,
            tc.tile_pool(name="idxp", bufs=3) as ipool,
            tc.tile_pool(name="gat", bufs=3) as gpool,
            tc.tile_pool(name="mlp", bufs=2) as mpool,
            tc.tile_pool(name="sm", bufs=2) as smpool,
            tc.tile_pool(name="at", bufs=2) as apool,
            tc.tile_pool(name="ws", bufs=2) as wpool,
            tc.tile_pool(name="st", bufs=3) as stpool,
            tc.tile_pool(name="psz", bufs=1, space="PSUM") as psz_pool,
            tc.tile_pool(name="psb", bufs=2, space="PSUM") as psb_pool,
            tc.tile_pool(name="pss", bufs=1, space="PSUM") as pss_pool,
        ):
            ident = cpool.tile([128, 128], bf16)
            make_identity(nc, ident[:])
            # E[c, a*128+p] = (a == c): selector masks so bc_a = E_a.T @ attnT
            # replicates attnT row a across all 128 partitions
            emask = cpool.tile([N_ANC, N_ANC * 128], bf16)
            nc.gpsimd.memset(emask[:], 1.0)
            nc.gpsimd.affine_select(
                emask[:], emask[:], pattern=[[1, N_ANC * 128]],
                compare_op=mybir.AluOpType.is_ge, fill=0.0,
                base=0, channel_multiplier=-128)
            nc.gpsimd.affine_select(
                emask[:], emask[:], pattern=[[-1, N_ANC * 128]],
                compare_op=mybir.AluOpType.is_ge, fill=0.0,
                base=127, channel_multiplier=128)
            wl = cpool.tile([EMB, ATT], bf16)
            nc.sync.dma_start(wl[:], w_att[0:EMB, :])
            wa = cpool.tile([EMB, ATT], bf16)
            nc.sync.dma_start(wa[:], w_att[EMB:2 * EMB, :])
            bias = cpool.tile([ATT, 1], f32)
            nc.sync.dma_start(bias[:], b_att.rearrange("a b -> b a"))
            vv = cpool.tile([ATT, 1], bf16)
            nc.sync.dma_start(vv[:], v_att.rearrange("a b -> b a"))

            prev = None  # (gt-slice, mlp) of tile t-1
            gt2 = None

            for t in range(tiles + 1):
                if t < tiles and t % GATHER_TILES == 0:
                    g = t // GROUP_TILES
                    gi = t // GATHER_TILES
                    nidx = GATHER_TILES * NSLOT * 128
                    idx_sb = ipool.tile([128, gcols], i16, tag="idx")
                    nc.sync.dma_start(idx_sb[:], idx[gi])
                    gt2 = gpool.tile([128, GATHER_TILES * NSLOT * EMB], bf16,
                                     tag="gt")
                    nc.gpsimd.dma_gather(
                        out_ap=gt2[:].rearrange("p (one n) -> p one n", one=1),
                        in_ap=tab[g * TAB_ROWS:(g + 1) * TAB_ROWS, :],
                        idxs_ap=idx_sb[:],
                        num_idxs=nidx,
                        num_idxs_reg=nidx,
                        elem_size=EMB,
                        transpose=True,
                        single_packet=False,
                    )
                if t < tiles:
                    off = (t % GATHER_TILES) * NSLOT * EMB
                    gt = gt2[:, off:off + NSLOT * EMB]

                # --- stage 2 for tile t-1 ------------------------------
                if prev is not None:
                    pgt, pmlp = prev
                    s = t - 1
                    pre = pss_pool.tile([128, N_ANC], f32, tag="pre")
                    for j in range(N_ANC):
                        nc.tensor.matmul(pre[:, j:j + 1],
                                         lhsT=pmlp[:, j * ATT:(j + 1) * ATT],
                                         rhs=vv[:], start=True, stop=True)
                    ex = smpool.tile([128, N_ANC], f32, tag="ex")
                    ssum = smpool.tile([128, 1], f32, tag="ssum")
                    nc.scalar.activation(ex[:], pre[:], Act.Exp,
                                         accum_out=ssum[:])
                    rec = smpool.tile([128, 1], f32, tag="rec")
                    nc.vector.reciprocal(rec[:], ssum[:])
                    attn = smpool.tile([128, N_ANC], bf16, tag="attn")
                    nc.vector.tensor_mul(attn[:], ex[:],
                                         rec[:].to_broadcast([128, N_ANC]))
                    # attn.T -> [8, 128] so weighted sum can broadcast rows
                    pT = pss_pool.tile([N_ANC, 128], bf16, tag="pT")
                    nc.tensor.transpose(pT[:], attn[:], ident[:])
                    attnT = apool.tile([N_ANC, 128], bf16, tag="attnT")
                    nc.scalar.copy(attnT[:], pT[:])
                    # replicate each attn row across all 128 partitions via
                    # K=1 PE matmuls (DVE can't partition-broadcast)
                    bc = psb_pool.tile([128, N_ANC * 128], f32, tag="bc")
                    for a in range(N_ANC):
                        nc.tensor.matmul(bc[:, a * 128:(a + 1) * 128],
                                         lhsT=emask[:, a * 128:(a + 1) * 128],
                                         rhs=attnT[:], start=True, stop=True)
                    # weighted sum over ancestors, emb-major
                    ws = wpool.tile([128, N_ANC * EMB], bf16, tag="ws")
                    nc.vector.tensor_mul(ws[:], pgt[:, N_ANC * EMB:NSLOT * EMB],
                                         bc[:])
                    stage = stpool.tile([128, EMB], f32, tag="stage")
                    nc.vector.tensor_reduce(
                        stage[:], ws[:].rearrange("p (a n) -> p n a", a=N_ANC),
                        axis=mybir.AxisListType.X, op=mybir.AluOpType.add)
                    nc.sync.dma_start(outT[:, s * 128:(s + 1) * 128], stage[:])

                if t < tiles:
                    # --- z = W_l.T @ LT + W_a.T @ AT ----------------------
                    z = psz_pool.tile([128, N_ANC * ATT], f32, tag="z")
                    nc.tensor.matmul(z[:, 0:512], lhsT=wl[:], rhs=gt[:, 0:512],
                                     start=True, stop=False)
                    nc.tensor.matmul(z[:, 0:512], lhsT=wa[:], rhs=gt[:, 1024:1536],
                                     start=False, stop=True)
                    nc.tensor.matmul(z[:, 512:1024], lhsT=wl[:],
                                     rhs=gt[:, 512:1024], start=True, stop=False)
                    nc.tensor.matmul(z[:, 512:1024], lhsT=wa[:],
                                     rhs=gt[:, 1536:2048], start=False, stop=True)
                    mlp = mpool.tile([128, N_ANC * ATT], bf16, tag="mlp")
                    nc.scalar.activation(mlp[:, 0:512], z[:, 0:512], Act.Tanh,
                                         bias=bias[:])
                    nc.scalar.activation(mlp[:, 512:1024], z[:, 512:1024],
                                         Act.Tanh, bias=bias[:])
                    prev = (gt, mlp)

    nc.compile()
    return nc


def _get_nc(tiles=TILES, num_devices=NCORES):
    key = (tiles, num_devices)
    if key not in _nc_cache:
        _nc_cache[key] = _build(tiles, num_devices)
    return _nc_cache[key]


def _prep_core(ids_pad, W_bf16, tiles=TILES):
    """ids_pad: [tiles*128, 16] int32. Returns (tab, idx16) for one core."""
    groups = (tiles + GROUP_TILES - 1) // GROUP_TILES
    ngath = tiles // GATHER_TILES
    gcols = GATHER_TILES * 128 * NSLOT // 16
    tab = np.zeros((groups * TAB_ROWS, EMB), dtype=W_bf16.dtype)
    idx16 = np.zeros((ngath, 16, gcols), dtype=np.int16)
    for g in range(groups):
        t0, t1 = g * GROUP_TILES, min(tiles, (g + 1) * GROUP_TILES)
        codes = ids_pad[t0 * 128:t1 * 128]               # [nt*128, 16]
        nt = t1 - t0
        # gather order within tile: k = s*128 + n
        korder = codes.reshape(nt, 128, NSLOT).transpose(0, 2, 1)  # [nt, s, n]
        flat = korder.reshape(-1)                         # nt*2048, k-major
        uniq, inv = np.unique(flat, return_inverse=True)
        tab[g * TAB_ROWS:g * TAB_ROWS + len(uniq)] = W_bf16[uniq]
        inv = inv.astype(np.int16).reshape(nt // GATHER_TILES,
                                           GATHER_TILES * NSLOT * 128)
        for bi in range(nt // GATHER_TILES):
            # wrapped layout: idxs[k%16, k//16] = inv[k]
            gi = t0 // GATHER_TILES + bi
            idx16[gi] = inv[bi].reshape(gcols, 16).T      # [16, gcols]
    idx16 = np.broadcast_to(idx16[:, None, :, :],
                            (ngath, 8, 16, gcols)).reshape(ngath, 128, gcols)
    return tab, np.ascontiguousarray(idx16)


def _prep_in_maps(inputs, tiles=TILES):
    import ml_dtypes

    W_emb = np.ascontiguousarray(
        np.asarray(inputs["W_emb"], dtype=np.float32).astype(ml_dtypes.bfloat16))
    W_attention = np.ascontiguousarray(
        np.asarray(inputs["W_attention"], dtype=np.float32).astype(ml_dtypes.bfloat16))
    b_attention = np.ascontiguousarray(
        np.asarray(inputs["b_attention"], dtype=np.float32).reshape(1, ATT))
    v_attention = np.ascontiguousarray(
        np.asarray(inputs["v_attention"],
                   dtype=np.float32).astype(ml_dtypes.bfloat16).reshape(1, ATT))
    leaves = np.asarray(inputs["leaves"]).astype(np.int32)
    ancestors = np.asarray(inputs["ancestors"]).astype(np.int32)

    idx_all = np.concatenate([leaves, ancestors], axis=1)   # [N, 16]
    npad = tiles * 128
    in_maps = []
    for c in range(NCORES):
        shard = idx_all[c * NSH:(c + 1) * NSH]
        pad = np.zeros((npad, NSLOT), dtype=np.int32)
        pad[:NSH] = shard
        tab, idx16 = _prep_core(pad, W_emb, tiles)
        in_maps.append({
            "tab": tab,
            "w_att": W_attention,
            "b_att": b_attention,
            "v_att": v_attention,
            "idx": idx16,
        })
    return in_maps


def run(inputs, trace=False, **kwargs):
    """Run on the 8 NeuronCores; returns (output [N, E] f32, BassKernelResults)."""
    from concourse import bass_utils
    nc = _get_nc()
    in_maps = _prep_in_maps(inputs)
    res = bass_utils.run_bass_kernel_spmd(
        nc, in_maps, core_ids=list(range(NCORES)), trace=trace, **kwargs)
    outs = [res.results[c]["outT"][:, :NSH].T for c in range(NCORES)]
    full = np.concatenate(outs, axis=0).astype(np.float32)
    return full, res


def kernel(**inputs) -> np.ndarray:
    full, _ = run(inputs, trace=False)
    return full
